# revision 14
# baseline (speedup 1.0000x reference)
"""Trainium2 Bass kernel for an AttentionBlock (GroupNorm + single-head
spatial self-attention + residual), data-parallel over batch across 8
NeuronCores.  v2: fp8 DoubleRow matmuls + folded weights.

Math per sample (C=256, N=4096):
  xn = GroupNorm(x) * gn_w + gn_b
  s[i,j]  = (Wq xn_i + bq).(Wk xn_j + bk)/16
  out     = (Wp V softmax_j(s)) + bp + x,  V = Wv xn + bv

Folds (host):
  M   = Wk^T Wq            ->  sT[j,i] = xn_j^T (M xn_i + Wk^T bq) + f(i)
                               (f(i) is softmax-invariant, dropped)
  Wvp = 4 Wp Wv            ->  AV matmul directly produces the projected
                               output; bv lands in bp2 = bp + Wp bv
  xn8 = xn/4, g8 = (M xn + Wk^T bq)/4  ->  score psum = s/16 directly.

All heavy matmuls are fp8 (e4m3 operands; exp(scores) in e5m2) using
MatmulPerfMode.DoubleRow: [128, 2, F] operand tiles contract 256 deep at
0.5 cycles/row -- 2x the bf16/fp32r rate.

exp(scores) is split across engines: ACT runs real Exp; DVE approximates
exp directly in e5m2 bits (Schraudolph: bits = 4*log2(e)*s + 60.67,
float->uint8 convert, bit-viewed as e5m2).  Z = sum_j exp comes from an
all-ones DoubleRow matmul on the PE; normalization (U/Z) happens after
the (folded) projection, fused with bias+residual on Pool.
"""

import sys

sys.path.insert(0, "/opt/trn_rl_repo")

import numpy as np
import ml_dtypes

import concourse.bass as bass
import concourse.tile as tile
from concourse import mybir
from concourse.vector_clock import ScopedClock, VectorClock

# ---------------------------------------------------------------------------
# Workaround: this walrus build only accepts 1 sync-wait per instruction, but
# TileContext's final drain attaches one wait per live processor.  Emit one
# drain per processor instead.
# ---------------------------------------------------------------------------


def _patched_drain_and_barrier(self, tick_clock, wait_clock):
    gc = tick_clock.global_clock
    n = len(gc)
    for p in range(n):
        if gc[p] == 0:
            continue
        vec = [0] * n
        vec[p] = gc[p]
        nop = self.nc.sync.nop(nofuse=True, hint="tail_wait")
        wait_clock.add_sem_waits(nop.ins, ScopedClock({None: VectorClock(vec)}))
    self.nc.sync.drain()
    self.nc.all_engine_barrier()
    popped = self.nc._tile_sem_poison_stack.pop()
    assert popped is self._sem_poison
    self.nc.clear_and_free_semaphores(list(self.sems.allocated().values()))
    self.nc.all_engine_barrier()


tile.TileContext._drain_and_barrier = _patched_drain_and_barrier


# ---------------------------------------------------------------------------
# Same 1-wait-per-instruction constraint, applied globally: hoist excess
# sync-waits onto NoOps inserted immediately before the over-subscribed
# instruction (engines execute their stream in order, so this is identical).
# ---------------------------------------------------------------------------

import json as _json


def _split_excess_waits(bir_bytes: bytes) -> bytes:
    d = _json.loads(bir_bytes)
    changed = False
    for fn in d.get("functions", []):
        for bb in fn.get("blocks", []):
            out = []
            for ins in bb.get("instructions", []):
                si = ins.get("sync_info") or {}
                waits = si.get("on_wait") or []
                if len(waits) > 1 and "engine" in ins:
                    for i, w in enumerate(waits[:-1]):
                        out.append({
                            "engine": ins["engine"],
                            "ins": [],
                            "outs": [],
                            "name": f"{ins['name']}-xw{i}",
                            "opcode": "NoOp",
                            "sync_info": {"on_update": [], "on_wait": [w]},
                            "debug": ins.get("debug", 0),
                        })
                    si["on_wait"] = [waits[-1]]
                    changed = True
                out.append(ins)
            bb["instructions"] = out
    if not changed:
        return bir_bytes
    return _json.dumps(d).encode()


_orig_to_json_bytes = bass.Bass.to_json_bytes


def _patched_to_json_bytes(self):
    return _split_excess_waits(_orig_to_json_bytes(self))


bass.Bass.to_json_bytes = _patched_to_json_bytes

FP32 = mybir.dt.float32
FP32R = mybir.dt.float32r
BF16 = mybir.dt.bfloat16
E4 = mybir.dt.float8e4
E5 = mybir.dt.float8e5
U8 = mybir.dt.uint8
DR = mybir.MatmulPerfMode.DoubleRow

B = 8          # batch == number of cores
C = 256        # channels
H = W = 64
N = H * W      # 4096 spatial positions
G = 8          # groups
GS = C // G    # 32 channels per group
CB = 2         # channel blocks of 128
IC = 512       # i-chunk width
NI = N // IC   # 8 attention chunks
NP = N // 256  # 16 j-pairs (pair = 2 x 128-j-blocks)
EPS = 1e-5
INV_CNT = 1.0 / (GS * N)

# Schraudolph exp -> e5m2 bits: bits = SCH_A * s + SCH_B (float->uint8,
# truncating); covers s in [-10.4, 11.1] without clamping.
SCH_A = float(4.0 / np.log(2.0))
SCH_B = 60.0 + 0.172 + 0.5

Act = mybir.ActivationFunctionType
Alu = mybir.AluOpType


def build_bass(has_bp: bool = False):
    nc = bass.Bass()

    x_d = nc.declare_dram_parameter("xbf", [C, N], BF16, isOutput=False)
    wkq_d = nc.declare_dram_parameter("wkq8", [128, 2, C], E4, isOutput=False)
    wvp_d = nc.declare_dram_parameter("wvp8", [128, 2, C], E4, isOutput=False)
    bg_d = nc.declare_dram_parameter("bg4", [C, 1], FP32, isOutput=False)
    bp_d = nc.declare_dram_parameter("bp2", [C, 1], FP32, isOutput=False)
    gnw_d = nc.declare_dram_parameter("gnw4", [C, 1], FP32, isOutput=False)
    gnb_d = nc.declare_dram_parameter("gnb4", [C, 1], FP32, isOutput=False)
    gsel_d = nc.declare_dram_parameter("gsel", [C, G], FP32, isOutput=False)
    ones5_d = nc.declare_dram_parameter("ones5", [128, 2, 16], E5, isOutput=False)
    ones_row_d = nc.declare_dram_parameter("ones_row", [1, 128], FP32R, isOutput=False)
    ones_col_d = nc.declare_dram_parameter("ones_col", [128, 2], FP32R, isOutput=False)
    bpr_d = nc.declare_dram_parameter("bp_row", [1, C], FP32R, isOutput=False)
    bsel_d = nc.declare_dram_parameter("bsel", [G, C], FP32, isOutput=False)
    y_d = nc.declare_dram_parameter("y", [C, N], FP32, isOutput=True)

    with tile.TileContext(nc) as tc:
        with (
            nc.allow_low_precision(reason="fp8 attention"),
            tc.tile_pool(name="sb", bufs=1) as sb,
            tc.tile_pool(name="ps", bufs=1, space="PSUM") as ps,
        ):
            # ---------------- load x (critical path) ----------------------
            # split across both HWDGE queues (SP + ACT), 4 slabs per cb so
            # the stats pipeline starts on the first 1024 columns
            xs = [sb.tile([128, N], BF16, tag=f"x{cb}", name=f"x{cb}") for cb in range(CB)]
            XH = N // 4
            for h in range(4):
                for cb in range(CB):
                    eng = nc.sync if cb == 0 else nc.scalar
                    eng.dma_start(
                        out=xs[cb][:, h * XH : (h + 1) * XH],
                        in_=x_d[cb * 128 : (cb + 1) * 128, h * XH : (h + 1) * XH],
                    )

            # ---------------- weights / constants --------------------------
            wkq8 = sb.tile([128, 2, C], E4, tag="wkq8")
            wvp8 = sb.tile([128, 2, C], E4, tag="wvp8")
            nc.sync.dma_start(out=wkq8, in_=wkq_d[:, :, :])
            nc.sync.dma_start(out=wvp8, in_=wvp_d[:, :, :])

            bgt = [sb.tile([128, 1], FP32, tag=f"bg{cb}", name=f"bg{cb}") for cb in range(CB)]
            bpc = [sb.tile([128, 1], FP32, tag=f"bpc{cb}", name=f"bpc{cb}") for cb in range(CB)]
            gnw = [sb.tile([128, 1], FP32, tag=f"gnw{cb}", name=f"gnw{cb}") for cb in range(CB)]
            gnb = [sb.tile([128, 1], FP32, tag=f"gnb{cb}", name=f"gnb{cb}") for cb in range(CB)]
            gsel = [sb.tile([128, G], FP32, tag=f"gsel{cb}", name=f"gsel{cb}") for cb in range(CB)]
            for cb in range(CB):
                sl = slice(cb * 128, (cb + 1) * 128)
                nc.sync.dma_start(out=bgt[cb], in_=bg_d[sl, :])
                nc.sync.dma_start(out=bpc[cb], in_=bp_d[sl, :])
                nc.sync.dma_start(out=gnw[cb], in_=gnw_d[sl, :])
                nc.sync.dma_start(out=gnb[cb], in_=gnb_d[sl, :])
                nc.sync.dma_start(out=gsel[cb], in_=gsel_d[sl, :])
            bsel = sb.tile([G, C], FP32, tag="bsel")
            nc.sync.dma_start(out=bsel, in_=bsel_d[:, :])

            # 1.0-filled e5m2 tile for the Z (sum_j exp) DoubleRow matmul.
            # Dual-fp8 LDWEIGHTS needs the k-pair stride 16B-aligned, so the
            # tile is [128, 2, 16] and the matmul uses [:, :, 0:2] (M=2).
            # DMA'd from DRAM: walrus rejects memsets of 8/16-bit int views.
            ones5 = sb.tile([128, 2, 16], E5, tag="ones5")
            nc.sync.dma_start(out=ones5, in_=ones5_d[:, :, :])
            ones_row = sb.tile([1, 128], FP32R, tag="ones_row")
            nc.sync.dma_start(out=ones_row, in_=ones_row_d[:, :])
            ones_col = sb.tile([128, 2], FP32R, tag="ones_col")
            nc.sync.dma_start(out=ones_col, in_=ones_col_d[:, :])
            bp_row = sb.tile([1, C], FP32R, tag="bp_row")
            nc.sync.dma_start(out=bp_row, in_=bpr_d[:, :])

            # PE observes static-tile producers early so real matmuls need
            # at most one sync wait (walrus limit); excess waits are NoOp-
            # hoisted by _split_excess_waits anyway.
            def pe_touch(ap):
                # always view as bf16: fp8 ldweights trips the dual-fp8 ISA
                # restrictions and 4-byte dtypes are refused outright
                if mybir.dt.size(ap.dtype) != 2:
                    ap = ap.bitcast(mybir.dt.bfloat16)
                sl = [slice(0, 1)] * len(ap.shape)
                for d in range(len(ap.shape) - 1, 0, -1):
                    if ap.shape[d] >= 2:
                        sl[d] = slice(0, 2)
                        break
                nc.tensor.ldweights(ap[tuple(sl)])

            for t in (wkq8, wvp8, ones5):
                pe_touch(t)
            for t in (gsel[0], gsel[1], bsel, ones_row, ones_col, bp_row):
                pe_touch(t)

            # Let the DVE observe the small-constant DMA queues early.
            for t in (gnw[0], gnw[1], gnb[0], gnb[1]):
                dvt = sb.tile([128, 1], FP32, tag="dvt", bufs=1, name="dvt")
                nc.vector.tensor_copy(out=dvt, in_=t)

            # ---------------- group-norm statistics ------------------------
            stat = [sb.tile([128, 2], FP32, tag=f"stat{cb}", name=f"stat{cb}") for cb in range(CB)]
            SQCH = 1024
            sums = [sb.tile([128, 4], FP32, tag=f"sums{cb}", bufs=1, name="sums") for cb in range(CB)]
            sqas = [sb.tile([128, N // SQCH], FP32, tag=f"sqa{cb}", bufs=1, name="sqa") for cb in range(CB)]
            for h in range(4):
                for cb in range(CB):
                    nc.vector.reduce_sum(
                        sums[cb][:, h : h + 1],
                        xs[cb][:, h * XH : (h + 1) * XH],
                        axis=mybir.AxisListType.X,
                    )
                    scr = sb.tile([128, SQCH], FP32, tag="sq_scratch", bufs=2, name="scr")
                    nc.scalar.activation(
                        out=scr, in_=xs[cb][:, h * SQCH : (h + 1) * SQCH],
                        func=Act.Square, accum_out=sqas[cb][:, h : h + 1],
                    )
            for cb in range(CB):
                nc.vector.reduce_sum(stat[cb][:, 0:1], sums[cb], axis=mybir.AxisListType.X)
                nc.vector.reduce_sum(stat[cb][:, 1:2], sqas[cb], axis=mybir.AxisListType.X)

            gstats_ps = ps.tile([G, 2], FP32, tag="pp", bufs=3, name="gstats_ps")
            for cb in range(CB):
                nc.tensor.matmul(
                    gstats_ps, lhsT=gsel[cb], rhs=stat[cb],
                    start=(cb == 0), stop=(cb == CB - 1),
                )
            m2 = sb.tile([G, 2], FP32, tag="m2")
            nc.vector.tensor_scalar_mul(out=m2, in0=gstats_ps, scalar1=INV_CNT)
            meansq = sb.tile([G, 1], FP32, tag="meansq")
            nc.vector.tensor_mul(out=meansq, in0=m2[:, 0:1], in1=m2[:, 0:1])
            gm = sb.tile([G, 2], FP32, tag="gm")
            nc.vector.tensor_sub(out=gm[:, 1:2], in0=m2[:, 1:2], in1=meansq)
            eps_t = sb.tile([G, 1], FP32, tag="eps_t")
            nc.vector.memset(eps_t, EPS)
            nc.scalar.activation(out=gm[:, 1:2], in_=gm[:, 1:2], func=Act.Sqrt, bias=eps_t)
            nc.vector.reciprocal(out=gm[:, 1:2], in_=gm[:, 1:2])
            nc.vector.tensor_copy(out=gm[:, 0:1], in_=m2[:, 0:1])
            pe_touch(gm)

            scale_v = []
            bias_v = []
            for cb in range(CB):
                bvals_ps = ps.tile([128, 2], FP32, tag="pp", bufs=3, name="bvals_ps")
                nc.tensor.matmul(
                    bvals_ps, lhsT=bsel[:, cb * 128 : (cb + 1) * 128], rhs=gm,
                    start=True, stop=True,
                )
                sc = sb.tile([128, 1], FP32, tag=f"scale{cb}", name=f"scale{cb}")
                bi = sb.tile([128, 1], FP32, tag=f"bias{cb}", name=f"bias{cb}")
                tmp = sb.tile([128, 1], FP32, tag=f"tmpb{cb}", name=f"tmpb{cb}")
                # sc = rstd * gn_w/4 ; bi = gn_b/4 - mean * sc
                nc.vector.tensor_mul(out=sc, in0=bvals_ps[:, 1:2], in1=gnw[cb])
                nc.vector.tensor_mul(out=tmp, in0=bvals_ps[:, 0:1], in1=sc)
                nc.vector.tensor_sub(out=bi, in0=gnb[cb], in1=tmp)
                scale_v.append(sc)
                bias_v.append(bi)

            # ---------------- xn8 / g8 / vp (phase B) ----------------------
            xn8 = sb.tile([128, 2, N], E4, tag="xn8")
            g8 = sb.tile([128, 2, N], E4, tag="g8")
            vpp = [
                sb.tile([128, 2, C], E4, tag="vpp", bufs=NP, name=f"vpp{m}")
                for m in range(NP)
            ]

            BC = 1024  # big-chunk width for phase B
            for bc in range(N // BC):
                nsl = slice(bc * BC, (bc + 1) * BC)
                # xn8 = x*sc + bi: cb0 on ACT (Identity), cb1 on DVE
                # (tensor_scalar) so the halves run in parallel; Pool can't
                # help -- TensorScalarPtr is not a valid Pool opcode.
                nc.scalar.activation(
                    out=xn8[:, 0, nsl], in_=xs[0][:, nsl], func=Act.Identity,
                    bias=bias_v[0], scale=scale_v[0],
                )
                nc.vector.tensor_scalar(
                    out=xn8[:, 1, nsl], in0=xs[1][:, nsl],
                    scalar1=scale_v[1], scalar2=bias_v[1],
                    op0=Alu.mult, op1=Alu.add,
                )
                # g = M xn + bg  (one DR matmul + conv per 512-half, out of
                # the 1-bank "pp" ring; convs alternate ACT/DVE)
                for ob in range(CB):
                    osl = slice(ob * 128, (ob + 1) * 128)
                    for hh in range(2):
                        hsl = slice(bc * BC + hh * IC, bc * BC + (hh + 1) * IC)
                        gp = ps.tile([128, IC], FP32, tag="pp", bufs=3, name="gp")
                        nc.tensor.matmul(
                            gp, lhsT=wkq8[:, :, osl], rhs=xn8[:, :, hsl],
                            start=True, stop=True, perf_mode=DR,
                        )
                        if (ob + hh) % 2 == 0:
                            nc.scalar.activation(
                                out=g8[:, ob, hsl], in_=gp,
                                func=Act.Identity, bias=bgt[ob],
                            )
                        else:
                            nc.vector.tensor_scalar_add(
                                out=g8[:, ob, hsl], in0=gp, scalar1=bgt[ob],
                            )
                # vp = Wvp4 xn8 per 128-j block; pairs packed for DR AV
                for mm_i in range(4):
                    m = bc * 4 + mm_i
                    for i2 in range(2):
                        jb = 2 * m + i2
                        jsl = slice(jb * 128, (jb + 1) * 128)
                        vpm = ps.tile([128, C], FP32, tag="pp", bufs=3, name="vpm")
                        nc.tensor.matmul(
                            vpm, lhsT=xn8[:, :, jsl], rhs=wvp8,
                            start=True, stop=True, perf_mode=DR,
                        )
                        if (mm_i + i2) % 2 == 0:
                            nc.vector.tensor_copy(out=vpp[m][:, i2, :], in_=vpm)
                        else:
                            nc.scalar.copy(out=vpp[m][:, i2, :], in_=vpm)

            # ---------------- attention (phase C) --------------------------
            LAG = 2
            pending = []
            for ich in range(NI):
                isl = slice(ich * IC, (ich + 1) * IC)

                pp_ps = [
                    ps.tile([128, IC], FP32, tag="pp", bufs=3, name=f"pp{cb}_{ich}")
                    for cb in range(CB)
                ]
                z_ps = ps.tile([2, IC], FP32, tag="z", bufs=1, name=f"z{ich}")

                ets = [None] * NP
                zpp = sb.tile([128, 2, IC], FP32R, tag="zpp", bufs=2, name="zpp")

                def issue_st(m):
                    # single-bank score tiles in a 5-deep ring: the PE can
                    # run 2.5 pairs ahead of the exp engines instead of 2
                    et = sb.tile([128, 2, IC], E5, tag="et", bufs=6, name=f"et{m}")
                    et_u8 = et.bitcast(U8)
                    for i2 in range(2):
                        jb = 2 * m + i2
                        jsl = slice(jb * 128, (jb + 1) * 128)
                        stp = ps.tile([128, IC], FP32, tag="mm", bufs=4, name="stp")
                        nc.tensor.matmul(
                            stp, lhsT=xn8[:, :, jsl], rhs=g8[:, :, isl],
                            start=True, stop=True, perf_mode=DR,
                        )
                        # alternate which engine takes which half per pair;
                        # pair 5 goes fully to DVE (ACT 15 / DVE 17 balance:
                        # ACT also carries the zs/ppc/zbs tail copies)
                        if (m + i2) % 2 == 0 and m != 5:
                            nc.scalar.activation(
                                out=et[:, i2, :], in_=stp, func=Act.Exp,
                            )
                        else:
                            nc.vector.tensor_scalar(
                                out=et_u8[:, i2, :], in0=stp,
                                scalar1=SCH_A, scalar2=SCH_B,
                                op0=Alu.mult, op1=Alu.add,
                            )
                    ets[m] = et

                PZ = (2, 6, 10, 14)  # pairs whose Z partials go to Pool

                def issue_av(m):
                    et = ets[m]
                    for cb in range(CB):
                        # with bias, the pp group is closed by the bp*Z matmul
                        nc.tensor.matmul(
                            pp_ps[cb], lhsT=vpp[m][:, :, cb * 128 : (cb + 1) * 128],
                            rhs=et, start=(m == 0),
                            stop=(not has_bp and m == NP - 1),
                            perf_mode=DR,
                        )
                    if m in PZ:
                        # Pool accumulates these pairs' exp sums in SBUF;
                        # folded into z_ps by the tail's ones-column matmuls
                        if m == PZ[0]:
                            nc.gpsimd.tensor_copy(out=zpp, in_=et)
                        else:
                            nc.gpsimd.tensor_add(out=zpp, in0=zpp, in1=et)
                    else:
                        # the z group is closed by the tail's fold matmuls
                        nc.tensor.matmul(
                            z_ps, lhsT=ones5[:, :, 0:2], rhs=et,
                            start=(m == 0), stop=False, perf_mode=DR,
                        )

                for m in range(NP + LAG):
                    for fm, fn in pending:
                        if fm == m:
                            fn()
                    if m < NP:
                        issue_st(m)
                    if m >= LAG:
                        issue_av(m - LAG)
                pending = []

                last = ich == NI - 1

                def make_tails(ich=ich, isl=isl, pp_ps=pp_ps, zpp=zpp,
                               z_ps=z_ps, last=last):
                    state = {}

                    def tail_early():
                        # fold the Pool-accumulated Z partials across
                        # partitions; these close the z accumulation group
                        for blk in range(2):
                            nc.tensor.matmul(
                                z_ps, lhsT=ones_col, rhs=zpp[:, blk, :],
                                start=False, stop=(blk == 1),
                            )
                        # Z copy out of PSUM; reciprocal runs on a DMA-
                        # reshaped [128, 4] view so the 6-pass DVE reciprocal
                        # costs ~0.2us instead of 3us on [1, 512].  The last
                        # chunk takes the direct lower-latency reciprocal.
                        zs = sb.tile([1, IC], FP32R, tag="zs", bufs=2, name="zs")
                        nc.scalar.copy(out=zs, in_=z_ps[0:1, :])
                        state["zs"] = zs
                        if not last:
                            zt = sb.tile([128, 4], FP32R, tag="zt", bufs=2, name="zt")
                            nc.sync.dma_start(out=zt, in_=zs)
                            state["zt"] = zt
                        # bias (when nonzero) enters pre-normalization:
                        # pp += bp * Z, so pp/Z carries +bp.  These rank-1
                        # fp32r matmuls also close the pp accumulation groups.
                        if has_bp:
                            for ob in range(CB):
                                nc.tensor.matmul(
                                    pp_ps[ob],
                                    lhsT=bp_row[:, ob * 128 : (ob + 1) * 128],
                                    rhs=state["zs"], start=False, stop=True,
                                )
                        # only ppc0 must run this early: it frees the pp ring
                        # slot the next chunk's second accumulator lands in.
                        # ppc1's slot isn't needed for another full chunk, so
                        # it runs at tail_mid, keeping ACT's exp cadence.
                        ppc = sb.tile([128, IC], FP32, tag="ppc", bufs=3, name="ppc")
                        nc.scalar.copy(out=ppc, in_=pp_ps[0])
                        state["ppcs"] = [ppc]

                    def tail_mid():
                        ppc = sb.tile([128, IC], FP32, tag="ppc", bufs=3, name="ppc")
                        nc.scalar.copy(out=ppc, in_=pp_ps[1])
                        state["ppcs"].append(ppc)

                    def tail_recip():
                        # placed a few pairs into the next chunk so the
                        # zs->zt DMA has landed and DVE doesn't stall
                        zrr = sb.tile([1, IC], FP32R, tag="zrr", bufs=2, name="zrr")
                        if last:
                            nc.vector.reciprocal(out=zrr, in_=z_ps[0:1, :])
                        else:
                            ztr = sb.tile([128, 4], FP32R, tag="ztr", bufs=2, name="ztr")
                            nc.vector.reciprocal(out=ztr, in_=state["zt"])
                            nc.sync.dma_start(out=zrr, in_=ztr)
                        state["zrr"] = zrr

                    def tail_late():
                        zb_ps = ps.tile([128, IC], FP32, tag="mm", bufs=4, name="zb")
                        nc.tensor.matmul(
                            zb_ps, lhsT=ones_row, rhs=state["zrr"],
                            start=True, stop=True,
                        )
                        zbs = sb.tile([128, IC], FP32, tag="zbs", bufs=2, name="zbs")
                        nc.scalar.copy(out=zbs, in_=zb_ps)
                        for ob in range(CB):
                            osl = slice(ob * 128, (ob + 1) * 128)
                            t = sb.tile([128, IC], FP32, tag="tn", bufs=2, name="tn")
                            nc.gpsimd.tensor_mul(out=t, in0=state["ppcs"][ob], in1=zbs)
                            fin = sb.tile([128, IC], FP32, tag="fin", bufs=3, name="fin")
                            nc.gpsimd.tensor_add(out=fin, in0=t, in1=xs[ob][:, isl])
                            nc.sync.dma_start(out=y_d[osl, isl], in_=fin)

                    return [(0, tail_early), (3, tail_recip), (4, tail_mid),
                            (6, tail_late)]

                pending = make_tails()
            for _, fn in pending:
                fn()

    return nc


def _prep_inputs(x_full, gn_w, gn_b, wq, bq, wk, bk, wv, bv, wp, bp):
    f = np.float32
    f64 = np.float64
    M = (np.asarray(wk, f64).T @ np.asarray(wq, f64)).astype(f)
    Wvp4 = (4.0 * (np.asarray(wp, f64) @ np.asarray(wv, f64))).astype(f)
    bg4 = ((np.asarray(wk, f64).T @ np.asarray(bq, f64)) / 4.0).astype(f).reshape(C, 1)
    bp2 = (np.asarray(bp, f64) + np.asarray(wp, f64) @ np.asarray(bv, f64)
           ).astype(f).reshape(C, 1)

    def dr_pack(mat):
        # [C, C] weight (contraction dim first) -> [128, 2, C] DoubleRow tile
        return np.ascontiguousarray(
            mat.reshape(2, 128, C).transpose(1, 0, 2)
        ).astype(ml_dtypes.float8_e4m3)

    # g[o, n] = sum_c M[o, c] xn[c, n]  ->  lhsT[p, blk, o] = M.T[blk*128+p, o]
    wkq8 = dr_pack(np.ascontiguousarray(M.T))
    wvp8 = dr_pack(np.ascontiguousarray(Wvp4.T))

    gnw4 = (np.asarray(gn_w, f) / 4.0).reshape(C, 1)
    gnb4 = (np.asarray(gn_b, f) / 4.0).reshape(C, 1)
    gsel = np.zeros((C, G), f)
    for c in range(C):
        gsel[c, c // GS] = 1.0
    bsel = np.ascontiguousarray(gsel.T)

    shared = dict(
        wkq8=wkq8, wvp8=wvp8, bg4=bg4, bp2=bp2,
        gnw4=gnw4, gnb4=gnb4, gsel=gsel, bsel=bsel,
        ones5=np.ones((128, 2, 16), ml_dtypes.float8_e5m2),
        ones_row=np.ones((1, 128), f),
        ones_col=np.ones((128, 2), f),
        bp_row=np.ascontiguousarray(bp2.reshape(1, C)),
    )
    in_maps = []
    for b in range(B):
        m = dict(shared)
        m["xbf"] = np.ascontiguousarray(
            x_full[b].reshape(C, N).astype(ml_dtypes.bfloat16)
        )
        in_maps.append(m)
    return in_maps


_CACHED_NC = {}


def _get_nc(has_bp: bool = False):
    if has_bp not in _CACHED_NC:
        _CACHED_NC[has_bp] = build_bass(has_bp)
    return _CACHED_NC[has_bp]


def kernel(x, gn_w, gn_b, wq, bq, wk, bk, wv, bv, wp, bp):
    from concourse.bass_utils import run_bass_kernel_spmd

    in_maps = _prep_inputs(
        np.asarray(x), np.asarray(gn_w), np.asarray(gn_b),
        np.asarray(wq), np.asarray(bq), np.asarray(wk), np.asarray(bk),
        np.asarray(wv), np.asarray(bv), np.asarray(wp), np.asarray(bp),
    )
    nc = _get_nc(has_bp=bool(np.any(in_maps[0]["bp_row"])))
    res = run_bass_kernel_spmd(nc, in_maps, list(range(B)))
    out = np.empty((B, C, H, W), np.float32)
    for b in range(B):
        out[b] = res.results[b]["y"].reshape(C, H, W)
    return out


# revision 15
# speedup vs baseline: 1.2053x; 1.2053x over previous
"""Trainium2 Bass kernel for an AttentionBlock (GroupNorm + single-head
spatial self-attention + residual), data-parallel over batch across 8
NeuronCores.  v2: fp8 DoubleRow matmuls + folded weights.

Math per sample (C=256, N=4096):
  xn = GroupNorm(x) * gn_w + gn_b
  s[i,j]  = (Wq xn_i + bq).(Wk xn_j + bk)/16
  out     = (Wp V softmax_j(s)) + bp + x,  V = Wv xn + bv

Folds (host):
  M   = Wk^T Wq            ->  sT[j,i] = xn_j^T (M xn_i + Wk^T bq) + f(i)
                               (f(i) is softmax-invariant, dropped)
  Wvp = 4 Wp Wv            ->  AV matmul directly produces the projected
                               output; bv lands in bp2 = bp + Wp bv
  xn8 = xn/4, g8 = (M xn + Wk^T bq)/4  ->  score psum = s/16 directly.

All heavy matmuls are fp8 (e4m3 operands; exp(scores) in e5m2) using
MatmulPerfMode.DoubleRow: [128, 2, F] operand tiles contract 256 deep at
0.5 cycles/row -- 2x the bf16/fp32r rate.

exp(scores) is split across engines: ACT runs real Exp; DVE approximates
exp directly in e5m2 bits (Schraudolph: bits = 4*log2(e)*s + 60.67,
float->uint8 convert, bit-viewed as e5m2).  Z = sum_j exp comes from an
all-ones DoubleRow matmul on the PE; normalization (U/Z) happens after
the (folded) projection, fused with bias+residual on Pool.
"""

import sys

sys.path.insert(0, "/opt/trn_rl_repo")

import numpy as np
import ml_dtypes

import concourse.bass as bass
import concourse.tile as tile
from concourse import mybir
from concourse.vector_clock import ScopedClock, VectorClock

# ---------------------------------------------------------------------------
# Workaround: this walrus build only accepts 1 sync-wait per instruction, but
# TileContext's final drain attaches one wait per live processor.  Emit one
# drain per processor instead.
# ---------------------------------------------------------------------------


def _patched_drain_and_barrier(self, tick_clock, wait_clock):
    gc = tick_clock.global_clock
    n = len(gc)
    for p in range(n):
        if gc[p] == 0:
            continue
        vec = [0] * n
        vec[p] = gc[p]
        nop = self.nc.sync.nop(nofuse=True, hint="tail_wait")
        wait_clock.add_sem_waits(nop.ins, ScopedClock({None: VectorClock(vec)}))
    self.nc.sync.drain()
    self.nc.all_engine_barrier()
    popped = self.nc._tile_sem_poison_stack.pop()
    assert popped is self._sem_poison
    self.nc.clear_and_free_semaphores(list(self.sems.allocated().values()))
    self.nc.all_engine_barrier()


tile.TileContext._drain_and_barrier = _patched_drain_and_barrier


# ---------------------------------------------------------------------------
# Same 1-wait-per-instruction constraint, applied globally: hoist excess
# sync-waits onto NoOps inserted immediately before the over-subscribed
# instruction (engines execute their stream in order, so this is identical).
# ---------------------------------------------------------------------------

import json as _json


def _split_excess_waits(bir_bytes: bytes) -> bytes:
    d = _json.loads(bir_bytes)
    changed = False
    for fn in d.get("functions", []):
        for bb in fn.get("blocks", []):
            out = []
            for ins in bb.get("instructions", []):
                si = ins.get("sync_info") or {}
                waits = si.get("on_wait") or []
                if len(waits) > 1 and "engine" in ins:
                    for i, w in enumerate(waits[:-1]):
                        out.append({
                            "engine": ins["engine"],
                            "ins": [],
                            "outs": [],
                            "name": f"{ins['name']}-xw{i}",
                            "opcode": "NoOp",
                            "sync_info": {"on_update": [], "on_wait": [w]},
                            "debug": ins.get("debug", 0),
                        })
                    si["on_wait"] = [waits[-1]]
                    changed = True
                out.append(ins)
            bb["instructions"] = out
    if not changed:
        return bir_bytes
    return _json.dumps(d).encode()


_orig_to_json_bytes = bass.Bass.to_json_bytes


def _patched_to_json_bytes(self):
    return _split_excess_waits(_orig_to_json_bytes(self))


bass.Bass.to_json_bytes = _patched_to_json_bytes

FP32 = mybir.dt.float32
FP32R = mybir.dt.float32r
BF16 = mybir.dt.bfloat16
E4 = mybir.dt.float8e4
E5 = mybir.dt.float8e5
U8 = mybir.dt.uint8
DR = mybir.MatmulPerfMode.DoubleRow

B = 8          # batch == number of cores
C = 256        # channels
H = W = 64
N = H * W      # 4096 spatial positions
G = 8          # groups
GS = C // G    # 32 channels per group
CB = 2         # channel blocks of 128
IC = 512       # i-chunk width
NI = N // IC   # 8 attention chunks
NP = N // 256  # 16 j-pairs (pair = 2 x 128-j-blocks)
EPS = 1e-5
INV_CNT = 1.0 / (GS * N)

# Schraudolph exp -> e5m2 bits: bits = SCH_A * s + SCH_B (float->uint8,
# truncating); covers s in [-10.4, 11.1] without clamping.
SCH_A = float(4.0 / np.log(2.0))
SCH_B = 60.0 + 0.172 + 0.5

Act = mybir.ActivationFunctionType
Alu = mybir.AluOpType


def build_bass(has_bp: bool = False):
    nc = bass.Bass()

    x_d = nc.declare_dram_parameter("xbf", [C, N], BF16, isOutput=False)
    wkq_d = nc.declare_dram_parameter("wkq8", [128, 2, C], E4, isOutput=False)
    wvp_d = nc.declare_dram_parameter("wvp8", [128, 2, C], E4, isOutput=False)
    bg_d = nc.declare_dram_parameter("bg4", [C, 1], FP32, isOutput=False)
    bp_d = nc.declare_dram_parameter("bp2", [C, 1], FP32, isOutput=False)
    gnw_d = nc.declare_dram_parameter("gnw4", [C, 1], FP32, isOutput=False)
    gnb_d = nc.declare_dram_parameter("gnb4", [C, 1], FP32, isOutput=False)
    gsel_d = nc.declare_dram_parameter("gsel", [C, G], FP32, isOutput=False)
    ones5_d = nc.declare_dram_parameter("ones5", [128, 2, 16], E5, isOutput=False)
    ones_row_d = nc.declare_dram_parameter("ones_row", [1, 128], FP32R, isOutput=False)
    ones_col_d = nc.declare_dram_parameter("ones_col", [128, 2], FP32R, isOutput=False)
    bpr_d = nc.declare_dram_parameter("bp_row", [1, C], FP32R, isOutput=False)
    bsel_d = nc.declare_dram_parameter("bsel", [G, C], FP32, isOutput=False)
    y_d = nc.declare_dram_parameter("y", [C, N], FP32, isOutput=True)

    with tile.TileContext(nc) as tc:
        with (
            nc.allow_low_precision(reason="fp8 attention"),
            tc.tile_pool(name="sb", bufs=1) as sb,
            tc.tile_pool(name="ps", bufs=1, space="PSUM") as ps,
        ):
            # ---------------- load x (critical path) ----------------------
            # split across both HWDGE queues (SP + ACT), 4 slabs per cb so
            # the stats pipeline starts on the first 1024 columns
            xs = [sb.tile([128, N], BF16, tag=f"x{cb}", name=f"x{cb}") for cb in range(CB)]
            XH = N // 4
            for h in range(4):
                for cb in range(CB):
                    eng = nc.sync if cb == 0 else nc.scalar
                    eng.dma_start(
                        out=xs[cb][:, h * XH : (h + 1) * XH],
                        in_=x_d[cb * 128 : (cb + 1) * 128, h * XH : (h + 1) * XH],
                    )

            # ---------------- weights / constants --------------------------
            wkq8 = sb.tile([128, 2, C], E4, tag="wkq8")
            wvp8 = sb.tile([128, 2, C], E4, tag="wvp8")
            nc.sync.dma_start(out=wkq8, in_=wkq_d[:, :, :])
            nc.sync.dma_start(out=wvp8, in_=wvp_d[:, :, :])

            bgt = [sb.tile([128, 1], FP32, tag=f"bg{cb}", name=f"bg{cb}") for cb in range(CB)]
            bpc = [sb.tile([128, 1], FP32, tag=f"bpc{cb}", name=f"bpc{cb}") for cb in range(CB)]
            gnw = [sb.tile([128, 1], FP32, tag=f"gnw{cb}", name=f"gnw{cb}") for cb in range(CB)]
            gnb = [sb.tile([128, 1], FP32, tag=f"gnb{cb}", name=f"gnb{cb}") for cb in range(CB)]
            gsel = [sb.tile([128, G], FP32, tag=f"gsel{cb}", name=f"gsel{cb}") for cb in range(CB)]
            for cb in range(CB):
                sl = slice(cb * 128, (cb + 1) * 128)
                nc.sync.dma_start(out=bgt[cb], in_=bg_d[sl, :])
                nc.sync.dma_start(out=bpc[cb], in_=bp_d[sl, :])
                nc.sync.dma_start(out=gnw[cb], in_=gnw_d[sl, :])
                nc.sync.dma_start(out=gnb[cb], in_=gnb_d[sl, :])
                nc.sync.dma_start(out=gsel[cb], in_=gsel_d[sl, :])
            bsel = sb.tile([G, C], FP32, tag="bsel")
            nc.sync.dma_start(out=bsel, in_=bsel_d[:, :])

            # 1.0-filled e5m2 tile for the Z (sum_j exp) DoubleRow matmul.
            # Dual-fp8 LDWEIGHTS needs the k-pair stride 16B-aligned, so the
            # tile is [128, 2, 16] and the matmul uses [:, :, 0:2] (M=2).
            # DMA'd from DRAM: walrus rejects memsets of 8/16-bit int views.
            ones5 = sb.tile([128, 2, 16], E5, tag="ones5")
            nc.sync.dma_start(out=ones5, in_=ones5_d[:, :, :])
            ones_row = sb.tile([1, 128], FP32R, tag="ones_row")
            nc.sync.dma_start(out=ones_row, in_=ones_row_d[:, :])
            ones_col = sb.tile([128, 2], FP32R, tag="ones_col")
            nc.sync.dma_start(out=ones_col, in_=ones_col_d[:, :])
            bp_row = sb.tile([1, C], FP32R, tag="bp_row")
            nc.sync.dma_start(out=bp_row, in_=bpr_d[:, :])

            # PE observes static-tile producers early so real matmuls need
            # at most one sync wait (walrus limit); excess waits are NoOp-
            # hoisted by _split_excess_waits anyway.
            def pe_touch(ap):
                # always view as bf16: fp8 ldweights trips the dual-fp8 ISA
                # restrictions and 4-byte dtypes are refused outright
                if mybir.dt.size(ap.dtype) != 2:
                    ap = ap.bitcast(mybir.dt.bfloat16)
                sl = [slice(0, 1)] * len(ap.shape)
                for d in range(len(ap.shape) - 1, 0, -1):
                    if ap.shape[d] >= 2:
                        sl[d] = slice(0, 2)
                        break
                nc.tensor.ldweights(ap[tuple(sl)])

            for t in (wkq8, wvp8, ones5):
                pe_touch(t)
            for t in (gsel[0], gsel[1], bsel, ones_row, ones_col, bp_row):
                pe_touch(t)

            # Let the DVE observe the small-constant DMA queues early.
            for t in (gnw[0], gnw[1], gnb[0], gnb[1]):
                dvt = sb.tile([128, 1], FP32, tag="dvt", bufs=1, name="dvt")
                nc.vector.tensor_copy(out=dvt, in_=t)

            # ---------------- group-norm statistics ------------------------
            stat = [sb.tile([128, 2], FP32, tag=f"stat{cb}", name=f"stat{cb}") for cb in range(CB)]
            SQCH = 1024
            sums = [sb.tile([128, 4], FP32, tag=f"sums{cb}", bufs=1, name="sums") for cb in range(CB)]
            sqas = [sb.tile([128, N // SQCH], FP32, tag=f"sqa{cb}", bufs=1, name="sqa") for cb in range(CB)]
            for h in range(4):
                for cb in range(CB):
                    nc.vector.reduce_sum(
                        sums[cb][:, h : h + 1],
                        xs[cb][:, h * XH : (h + 1) * XH],
                        axis=mybir.AxisListType.X,
                    )
                    scr = sb.tile([128, SQCH], FP32, tag="sq_scratch", bufs=2, name="scr")
                    nc.scalar.activation(
                        out=scr, in_=xs[cb][:, h * SQCH : (h + 1) * SQCH],
                        func=Act.Square, accum_out=sqas[cb][:, h : h + 1],
                    )
            for cb in range(CB):
                nc.vector.reduce_sum(stat[cb][:, 0:1], sums[cb], axis=mybir.AxisListType.X)
                nc.vector.reduce_sum(stat[cb][:, 1:2], sqas[cb], axis=mybir.AxisListType.X)

            gstats_ps = ps.tile([G, 2], FP32, tag="pp", bufs=3, name="gstats_ps")
            for cb in range(CB):
                nc.tensor.matmul(
                    gstats_ps, lhsT=gsel[cb], rhs=stat[cb],
                    start=(cb == 0), stop=(cb == CB - 1),
                )
            m2 = sb.tile([G, 2], FP32, tag="m2")
            nc.vector.tensor_scalar_mul(out=m2, in0=gstats_ps, scalar1=INV_CNT)
            meansq = sb.tile([G, 1], FP32, tag="meansq")
            nc.vector.tensor_mul(out=meansq, in0=m2[:, 0:1], in1=m2[:, 0:1])
            gm = sb.tile([G, 2], FP32, tag="gm")
            nc.vector.tensor_sub(out=gm[:, 1:2], in0=m2[:, 1:2], in1=meansq)
            eps_t = sb.tile([G, 1], FP32, tag="eps_t")
            nc.vector.memset(eps_t, EPS)
            nc.scalar.activation(out=gm[:, 1:2], in_=gm[:, 1:2], func=Act.Sqrt, bias=eps_t)
            nc.vector.reciprocal(out=gm[:, 1:2], in_=gm[:, 1:2])
            nc.vector.tensor_copy(out=gm[:, 0:1], in_=m2[:, 0:1])
            pe_touch(gm)

            scale_v = []
            bias_v = []
            for cb in range(CB):
                bvals_ps = ps.tile([128, 2], FP32, tag="pp", bufs=3, name="bvals_ps")
                nc.tensor.matmul(
                    bvals_ps, lhsT=bsel[:, cb * 128 : (cb + 1) * 128], rhs=gm,
                    start=True, stop=True,
                )
                sc = sb.tile([128, 1], FP32, tag=f"scale{cb}", name=f"scale{cb}")
                bi = sb.tile([128, 1], FP32, tag=f"bias{cb}", name=f"bias{cb}")
                tmp = sb.tile([128, 1], FP32, tag=f"tmpb{cb}", name=f"tmpb{cb}")
                # sc = rstd * gn_w/4 ; bi = gn_b/4 - mean * sc
                nc.vector.tensor_mul(out=sc, in0=bvals_ps[:, 1:2], in1=gnw[cb])
                nc.vector.tensor_mul(out=tmp, in0=bvals_ps[:, 0:1], in1=sc)
                nc.vector.tensor_sub(out=bi, in0=gnb[cb], in1=tmp)
                scale_v.append(sc)
                bias_v.append(bi)

            # ---------------- xn8 / g8 / vp (phase B) ----------------------
            xn8 = sb.tile([128, 2, N], E4, tag="xn8")
            g8 = sb.tile([128, 2, N], E4, tag="g8")
            vpp = [
                sb.tile([128, 2, C], E4, tag="vpp", bufs=NP, name=f"vpp{m}")
                for m in range(NP)
            ]

            BC = 1024  # big-chunk width for phase B
            for bc in range(N // BC):
                nsl = slice(bc * BC, (bc + 1) * BC)
                # xn8 = x*sc + bi: cb0 on ACT (Identity), cb1 on DVE
                # (tensor_scalar) so the halves run in parallel; Pool can't
                # help -- TensorScalarPtr is not a valid Pool opcode.
                nc.scalar.activation(
                    out=xn8[:, 0, nsl], in_=xs[0][:, nsl], func=Act.Identity,
                    bias=bias_v[0], scale=scale_v[0],
                )
                nc.vector.tensor_scalar(
                    out=xn8[:, 1, nsl], in0=xs[1][:, nsl],
                    scalar1=scale_v[1], scalar2=bias_v[1],
                    op0=Alu.mult, op1=Alu.add,
                )
                # g = M xn + bg  (one DR matmul + conv per 512-half, out of
                # the 1-bank "pp" ring; convs alternate ACT/DVE)
                for ob in range(CB):
                    osl = slice(ob * 128, (ob + 1) * 128)
                    for hh in range(2):
                        hsl = slice(bc * BC + hh * IC, bc * BC + (hh + 1) * IC)
                        gp = ps.tile([128, IC], FP32, tag="pp", bufs=3, name="gp")
                        nc.tensor.matmul(
                            gp, lhsT=wkq8[:, :, osl], rhs=xn8[:, :, hsl],
                            start=True, stop=True, perf_mode=DR,
                        )
                        if (ob + hh) % 2 == 0:
                            nc.scalar.activation(
                                out=g8[:, ob, hsl], in_=gp,
                                func=Act.Identity, bias=bgt[ob],
                            )
                        else:
                            nc.vector.tensor_scalar_add(
                                out=g8[:, ob, hsl], in0=gp, scalar1=bgt[ob],
                            )
                # vp = Wvp4 xn8 per 128-j block; pairs packed for DR AV
                for mm_i in range(4):
                    m = bc * 4 + mm_i
                    for i2 in range(2):
                        jb = 2 * m + i2
                        jsl = slice(jb * 128, (jb + 1) * 128)
                        vpm = ps.tile([128, C], FP32, tag="pp", bufs=3, name="vpm")
                        nc.tensor.matmul(
                            vpm, lhsT=xn8[:, :, jsl], rhs=wvp8,
                            start=True, stop=True, perf_mode=DR,
                        )
                        if (mm_i + i2) % 2 == 0:
                            nc.vector.tensor_copy(out=vpp[m][:, i2, :], in_=vpm)
                        else:
                            nc.scalar.copy(out=vpp[m][:, i2, :], in_=vpm)

            # ---------------- attention (phase C) --------------------------
            LAG = 2
            pending = []
            for ich in range(NI):
                isl = slice(ich * IC, (ich + 1) * IC)

                pp_ps = [
                    ps.tile([128, IC], FP32, tag="pp", bufs=3, name=f"pp{cb}_{ich}")
                    for cb in range(CB)
                ]
                z_ps = ps.tile([2, IC], FP32, tag="z", bufs=1, name=f"z{ich}")

                ets = [None] * NP

                def issue_st(m):
                    # single-bank score tiles in a 5-deep ring: the PE can
                    # run 2.5 pairs ahead of the exp engines instead of 2
                    et = sb.tile([128, 2, IC], E5, tag="et", bufs=6, name=f"et{m}")
                    et_u8 = et.bitcast(U8)
                    for i2 in range(2):
                        jb = 2 * m + i2
                        jsl = slice(jb * 128, (jb + 1) * 128)
                        stp = ps.tile([128, IC], FP32, tag="mm", bufs=4, name="stp")
                        nc.tensor.matmul(
                            stp, lhsT=xn8[:, :, jsl], rhs=g8[:, :, isl],
                            start=True, stop=True, perf_mode=DR,
                        )
                        # alternate which engine takes which half per pair;
                        # pair 5 goes fully to DVE (ACT 15 / DVE 17 balance:
                        # ACT also carries the zs/ppc/zbs tail copies)
                        if (m + i2) % 2 == 0 and m != 5:
                            nc.scalar.activation(
                                out=et[:, i2, :], in_=stp, func=Act.Exp,
                            )
                        else:
                            nc.vector.tensor_scalar(
                                out=et_u8[:, i2, :], in0=stp,
                                scalar1=SCH_A, scalar2=SCH_B,
                                op0=Alu.mult, op1=Alu.add,
                            )
                    ets[m] = et

                def issue_av(m):
                    et = ets[m]
                    for cb in range(CB):
                        # with bias, the pp group is closed by the bp*Z matmul
                        nc.tensor.matmul(
                            pp_ps[cb], lhsT=vpp[m][:, :, cb * 128 : (cb + 1) * 128],
                            rhs=et, start=(m == 0),
                            stop=(not has_bp and m == NP - 1),
                            perf_mode=DR,
                        )
                    nc.tensor.matmul(
                        z_ps, lhsT=ones5[:, :, 0:2], rhs=et,
                        start=(m == 0), stop=(m == NP - 1), perf_mode=DR,
                    )

                for m in range(NP + LAG):
                    for fm, fn in pending:
                        if fm == m:
                            fn()
                    if m < NP:
                        issue_st(m)
                    if m >= LAG:
                        issue_av(m - LAG)
                pending = []

                last = ich == NI - 1

                def make_tails(ich=ich, isl=isl, pp_ps=pp_ps,
                               z_ps=z_ps, last=last):
                    state = {}

                    def tail_early():
                        # Z copy out of PSUM; reciprocal runs on a DMA-
                        # reshaped [128, 4] view so the 6-pass DVE reciprocal
                        # costs ~0.2us instead of 3us on [1, 512].  The last
                        # chunk takes the direct lower-latency reciprocal.
                        zs = sb.tile([1, IC], FP32R, tag="zs", bufs=2, name="zs")
                        nc.scalar.copy(out=zs, in_=z_ps[0:1, :])
                        state["zs"] = zs
                        if not last:
                            zt = sb.tile([128, 4], FP32R, tag="zt", bufs=2, name="zt")
                            nc.sync.dma_start(out=zt, in_=zs)
                            state["zt"] = zt
                        # bias (when nonzero) enters pre-normalization:
                        # pp += bp * Z, so pp/Z carries +bp.  These rank-1
                        # fp32r matmuls also close the pp accumulation groups.
                        if has_bp:
                            for ob in range(CB):
                                nc.tensor.matmul(
                                    pp_ps[ob],
                                    lhsT=bp_row[:, ob * 128 : (ob + 1) * 128],
                                    rhs=state["zs"], start=False, stop=True,
                                )
                        # only ppc0 must run this early: it frees the pp ring
                        # slot the next chunk's second accumulator lands in.
                        # ppc1's slot isn't needed for another full chunk, so
                        # it runs at tail_mid, keeping ACT's exp cadence.
                        ppc = sb.tile([128, IC], FP32, tag="ppc", bufs=3, name="ppc")
                        nc.scalar.copy(out=ppc, in_=pp_ps[0])
                        state["ppcs"] = [ppc]

                    def tail_mid():
                        ppc = sb.tile([128, IC], FP32, tag="ppc", bufs=3, name="ppc")
                        nc.scalar.copy(out=ppc, in_=pp_ps[1])
                        state["ppcs"].append(ppc)

                    def tail_recip():
                        # placed a few pairs into the next chunk so the
                        # zs->zt DMA has landed and DVE doesn't stall
                        zrr = sb.tile([1, IC], FP32R, tag="zrr", bufs=2, name="zrr")
                        if last:
                            nc.vector.reciprocal(out=zrr, in_=z_ps[0:1, :])
                        else:
                            ztr = sb.tile([128, 4], FP32R, tag="ztr", bufs=2, name="ztr")
                            nc.vector.reciprocal(out=ztr, in_=state["zt"])
                            nc.sync.dma_start(out=zrr, in_=ztr)
                        state["zrr"] = zrr

                    def tail_late():
                        zb_ps = ps.tile([128, IC], FP32, tag="mm", bufs=4, name="zb")
                        nc.tensor.matmul(
                            zb_ps, lhsT=ones_row, rhs=state["zrr"],
                            start=True, stop=True,
                        )
                        zbs = sb.tile([128, IC], FP32, tag="zbs", bufs=2, name="zbs")
                        nc.scalar.copy(out=zbs, in_=zb_ps)
                        for ob in range(CB):
                            osl = slice(ob * 128, (ob + 1) * 128)
                            t = sb.tile([128, IC], FP32, tag="tn", bufs=2, name="tn")
                            nc.gpsimd.tensor_mul(out=t, in0=state["ppcs"][ob], in1=zbs)
                            fin = sb.tile([128, IC], FP32, tag="fin", bufs=3, name="fin")
                            nc.gpsimd.tensor_add(out=fin, in0=t, in1=xs[ob][:, isl])
                            nc.sync.dma_start(out=y_d[osl, isl], in_=fin)

                    return [(0, tail_early), (3, tail_recip), (4, tail_mid),
                            (6, tail_late)]

                pending = make_tails()
            for _, fn in pending:
                fn()

    return nc


def _prep_inputs(x_full, gn_w, gn_b, wq, bq, wk, bk, wv, bv, wp, bp):
    f = np.float32
    f64 = np.float64
    M = (np.asarray(wk, f64).T @ np.asarray(wq, f64)).astype(f)
    Wvp4 = (4.0 * (np.asarray(wp, f64) @ np.asarray(wv, f64))).astype(f)
    bg4 = ((np.asarray(wk, f64).T @ np.asarray(bq, f64)) / 4.0).astype(f).reshape(C, 1)
    bp2 = (np.asarray(bp, f64) + np.asarray(wp, f64) @ np.asarray(bv, f64)
           ).astype(f).reshape(C, 1)

    def dr_pack(mat):
        # [C, C] weight (contraction dim first) -> [128, 2, C] DoubleRow tile
        return np.ascontiguousarray(
            mat.reshape(2, 128, C).transpose(1, 0, 2)
        ).astype(ml_dtypes.float8_e4m3)

    # g[o, n] = sum_c M[o, c] xn[c, n]  ->  lhsT[p, blk, o] = M.T[blk*128+p, o]
    wkq8 = dr_pack(np.ascontiguousarray(M.T))
    wvp8 = dr_pack(np.ascontiguousarray(Wvp4.T))

    gnw4 = (np.asarray(gn_w, f) / 4.0).reshape(C, 1)
    gnb4 = (np.asarray(gn_b, f) / 4.0).reshape(C, 1)
    gsel = np.zeros((C, G), f)
    for c in range(C):
        gsel[c, c // GS] = 1.0
    bsel = np.ascontiguousarray(gsel.T)

    shared = dict(
        wkq8=wkq8, wvp8=wvp8, bg4=bg4, bp2=bp2,
        gnw4=gnw4, gnb4=gnb4, gsel=gsel, bsel=bsel,
        ones5=np.ones((128, 2, 16), ml_dtypes.float8_e5m2),
        ones_row=np.ones((1, 128), f),
        ones_col=np.ones((128, 2), f),
        bp_row=np.ascontiguousarray(bp2.reshape(1, C)),
    )
    in_maps = []
    for b in range(B):
        m = dict(shared)
        m["xbf"] = np.ascontiguousarray(
            x_full[b].reshape(C, N).astype(ml_dtypes.bfloat16)
        )
        in_maps.append(m)
    return in_maps


_CACHED_NC = {}


def _get_nc(has_bp: bool = False):
    if has_bp not in _CACHED_NC:
        _CACHED_NC[has_bp] = build_bass(has_bp)
    return _CACHED_NC[has_bp]


def kernel(x, gn_w, gn_b, wq, bq, wk, bk, wv, bv, wp, bp):
    from concourse.bass_utils import run_bass_kernel_spmd

    in_maps = _prep_inputs(
        np.asarray(x), np.asarray(gn_w), np.asarray(gn_b),
        np.asarray(wq), np.asarray(bq), np.asarray(wk), np.asarray(bk),
        np.asarray(wv), np.asarray(bv), np.asarray(wp), np.asarray(bp),
    )
    nc = _get_nc(has_bp=bool(np.any(in_maps[0]["bp_row"])))
    res = run_bass_kernel_spmd(nc, in_maps, list(range(B)))
    out = np.empty((B, C, H, W), np.float32)
    for b in range(B):
        out[b] = res.results[b]["y"].reshape(C, H, W)
    return out


# revision 16
# speedup vs baseline: 1.2133x; 1.0066x over previous
"""Trainium2 Bass kernel for an AttentionBlock (GroupNorm + single-head
spatial self-attention + residual), data-parallel over batch across 8
NeuronCores.  v2: fp8 DoubleRow matmuls + folded weights.

Math per sample (C=256, N=4096):
  xn = GroupNorm(x) * gn_w + gn_b
  s[i,j]  = (Wq xn_i + bq).(Wk xn_j + bk)/16
  out     = (Wp V softmax_j(s)) + bp + x,  V = Wv xn + bv

Folds (host):
  M   = Wk^T Wq            ->  sT[j,i] = xn_j^T (M xn_i + Wk^T bq) + f(i)
                               (f(i) is softmax-invariant, dropped)
  Wvp = 4 Wp Wv            ->  AV matmul directly produces the projected
                               output; bv lands in bp2 = bp + Wp bv
  xn8 = xn/4, g8 = (M xn + Wk^T bq)/4  ->  score psum = s/16 directly.

All heavy matmuls are fp8 (e4m3 operands; exp(scores) in e5m2) using
MatmulPerfMode.DoubleRow: [128, 2, F] operand tiles contract 256 deep at
0.5 cycles/row -- 2x the bf16/fp32r rate.

exp(scores) is split across engines: ACT runs real Exp; DVE approximates
exp directly in e5m2 bits (Schraudolph: bits = 4*log2(e)*s + 60.67,
float->uint8 convert, bit-viewed as e5m2).  Z = sum_j exp comes from an
all-ones DoubleRow matmul on the PE; normalization (U/Z) happens after
the (folded) projection, fused with bias+residual on Pool.
"""

import sys

sys.path.insert(0, "/opt/trn_rl_repo")

import numpy as np
import ml_dtypes

import concourse.bass as bass
import concourse.tile as tile
from concourse import mybir
from concourse.vector_clock import ScopedClock, VectorClock

# ---------------------------------------------------------------------------
# Workaround: this walrus build only accepts 1 sync-wait per instruction, but
# TileContext's final drain attaches one wait per live processor.  Emit one
# drain per processor instead.
# ---------------------------------------------------------------------------


def _patched_drain_and_barrier(self, tick_clock, wait_clock):
    gc = tick_clock.global_clock
    n = len(gc)
    for p in range(n):
        if gc[p] == 0:
            continue
        vec = [0] * n
        vec[p] = gc[p]
        nop = self.nc.sync.nop(nofuse=True, hint="tail_wait")
        wait_clock.add_sem_waits(nop.ins, ScopedClock({None: VectorClock(vec)}))
    self.nc.sync.drain()
    self.nc.all_engine_barrier()
    popped = self.nc._tile_sem_poison_stack.pop()
    assert popped is self._sem_poison
    self.nc.clear_and_free_semaphores(list(self.sems.allocated().values()))
    self.nc.all_engine_barrier()


tile.TileContext._drain_and_barrier = _patched_drain_and_barrier


# ---------------------------------------------------------------------------
# Same 1-wait-per-instruction constraint, applied globally: hoist excess
# sync-waits onto NoOps inserted immediately before the over-subscribed
# instruction (engines execute their stream in order, so this is identical).
# ---------------------------------------------------------------------------

import json as _json


def _split_excess_waits(bir_bytes: bytes) -> bytes:
    d = _json.loads(bir_bytes)
    changed = False
    for fn in d.get("functions", []):
        for bb in fn.get("blocks", []):
            out = []
            for ins in bb.get("instructions", []):
                si = ins.get("sync_info") or {}
                waits = si.get("on_wait") or []
                if len(waits) > 1 and "engine" in ins:
                    for i, w in enumerate(waits[:-1]):
                        out.append({
                            "engine": ins["engine"],
                            "ins": [],
                            "outs": [],
                            "name": f"{ins['name']}-xw{i}",
                            "opcode": "NoOp",
                            "sync_info": {"on_update": [], "on_wait": [w]},
                            "debug": ins.get("debug", 0),
                        })
                    si["on_wait"] = [waits[-1]]
                    changed = True
                out.append(ins)
            bb["instructions"] = out
    if not changed:
        return bir_bytes
    return _json.dumps(d).encode()


_orig_to_json_bytes = bass.Bass.to_json_bytes


def _patched_to_json_bytes(self):
    return _split_excess_waits(_orig_to_json_bytes(self))


bass.Bass.to_json_bytes = _patched_to_json_bytes

FP32 = mybir.dt.float32
FP32R = mybir.dt.float32r
BF16 = mybir.dt.bfloat16
E4 = mybir.dt.float8e4
E5 = mybir.dt.float8e5
U8 = mybir.dt.uint8
DR = mybir.MatmulPerfMode.DoubleRow

B = 8          # batch == number of cores
C = 256        # channels
H = W = 64
N = H * W      # 4096 spatial positions
G = 8          # groups
GS = C // G    # 32 channels per group
CB = 2         # channel blocks of 128
IC = 512       # i-chunk width
NI = N // IC   # 8 attention chunks
NP = N // 256  # 16 j-pairs (pair = 2 x 128-j-blocks)
EPS = 1e-5
INV_CNT = 1.0 / (GS * N)

# Schraudolph exp -> e5m2 bits: bits = SCH_A * s + SCH_B (float->uint8,
# truncating); covers s in [-10.4, 11.1] without clamping.
SCH_A = float(4.0 / np.log(2.0))
SCH_B = 60.0 + 0.172 + 0.5

Act = mybir.ActivationFunctionType
Alu = mybir.AluOpType


def build_bass(has_bp: bool = False):
    nc = bass.Bass()

    x_d = nc.declare_dram_parameter("xbf", [C, N], BF16, isOutput=False)
    wkq_d = nc.declare_dram_parameter("wkq8", [128, 2, C], E4, isOutput=False)
    wvp_d = nc.declare_dram_parameter("wvp8", [128, 2, C], E4, isOutput=False)
    bg_d = nc.declare_dram_parameter("bg4", [C, 1], FP32, isOutput=False)
    bp_d = nc.declare_dram_parameter("bp2", [C, 1], FP32, isOutput=False)
    gnw_d = nc.declare_dram_parameter("gnw4", [C, 1], FP32, isOutput=False)
    gnb_d = nc.declare_dram_parameter("gnb4", [C, 1], FP32, isOutput=False)
    gsel_d = nc.declare_dram_parameter("gsel", [C, G], FP32, isOutput=False)
    ones5_d = nc.declare_dram_parameter("ones5", [128, 2, 16], E5, isOutput=False)
    ones_row_d = nc.declare_dram_parameter("ones_row", [1, 128], FP32R, isOutput=False)
    ones_col_d = nc.declare_dram_parameter("ones_col", [128, 2], FP32R, isOutput=False)
    bpr_d = nc.declare_dram_parameter("bp_row", [1, C], FP32R, isOutput=False)
    bsel_d = nc.declare_dram_parameter("bsel", [G, C], FP32, isOutput=False)
    y_d = nc.declare_dram_parameter("y", [C, N], FP32, isOutput=True)

    with tile.TileContext(nc) as tc:
        with (
            nc.allow_low_precision(reason="fp8 attention"),
            tc.tile_pool(name="sb", bufs=1) as sb,
            tc.tile_pool(name="ps", bufs=1, space="PSUM") as ps,
        ):
            # ---------------- load x (critical path) ----------------------
            # split across both HWDGE queues (SP + ACT), 4 slabs per cb so
            # the stats pipeline starts on the first 1024 columns
            xs = [sb.tile([128, N], BF16, tag=f"x{cb}", name=f"x{cb}") for cb in range(CB)]
            XH = N // 4
            for h in range(4):
                for cb in range(CB):
                    eng = nc.sync if cb == 0 else nc.scalar
                    eng.dma_start(
                        out=xs[cb][:, h * XH : (h + 1) * XH],
                        in_=x_d[cb * 128 : (cb + 1) * 128, h * XH : (h + 1) * XH],
                    )

            # ---------------- weights / constants --------------------------
            wkq8 = sb.tile([128, 2, C], E4, tag="wkq8")
            wvp8 = sb.tile([128, 2, C], E4, tag="wvp8")
            nc.sync.dma_start(out=wkq8, in_=wkq_d[:, :, :])
            nc.sync.dma_start(out=wvp8, in_=wvp_d[:, :, :])

            bgt = [sb.tile([128, 1], FP32, tag=f"bg{cb}", name=f"bg{cb}") for cb in range(CB)]
            bpc = [sb.tile([128, 1], FP32, tag=f"bpc{cb}", name=f"bpc{cb}") for cb in range(CB)]
            gnw = [sb.tile([128, 1], FP32, tag=f"gnw{cb}", name=f"gnw{cb}") for cb in range(CB)]
            gnb = [sb.tile([128, 1], FP32, tag=f"gnb{cb}", name=f"gnb{cb}") for cb in range(CB)]
            gsel = [sb.tile([128, G], FP32, tag=f"gsel{cb}", name=f"gsel{cb}") for cb in range(CB)]
            for cb in range(CB):
                sl = slice(cb * 128, (cb + 1) * 128)
                nc.sync.dma_start(out=bgt[cb], in_=bg_d[sl, :])
                nc.sync.dma_start(out=bpc[cb], in_=bp_d[sl, :])
                nc.sync.dma_start(out=gnw[cb], in_=gnw_d[sl, :])
                nc.sync.dma_start(out=gnb[cb], in_=gnb_d[sl, :])
                nc.sync.dma_start(out=gsel[cb], in_=gsel_d[sl, :])
            bsel = sb.tile([G, C], FP32, tag="bsel")
            nc.sync.dma_start(out=bsel, in_=bsel_d[:, :])

            # 1.0-filled e5m2 tile for the Z (sum_j exp) DoubleRow matmul.
            # Dual-fp8 LDWEIGHTS needs the k-pair stride 16B-aligned, so the
            # tile is [128, 2, 16] and the matmul uses [:, :, 0:2] (M=2).
            # DMA'd from DRAM: walrus rejects memsets of 8/16-bit int views.
            ones5 = sb.tile([128, 2, 16], E5, tag="ones5")
            nc.sync.dma_start(out=ones5, in_=ones5_d[:, :, :])
            ones_row = sb.tile([1, 128], FP32R, tag="ones_row")
            nc.sync.dma_start(out=ones_row, in_=ones_row_d[:, :])
            ones_col = sb.tile([128, 2], FP32R, tag="ones_col")
            nc.sync.dma_start(out=ones_col, in_=ones_col_d[:, :])
            bp_row = sb.tile([1, C], FP32R, tag="bp_row")
            nc.sync.dma_start(out=bp_row, in_=bpr_d[:, :])

            # PE observes static-tile producers early so real matmuls need
            # at most one sync wait (walrus limit); excess waits are NoOp-
            # hoisted by _split_excess_waits anyway.
            def pe_touch(ap):
                # always view as bf16: fp8 ldweights trips the dual-fp8 ISA
                # restrictions and 4-byte dtypes are refused outright
                if mybir.dt.size(ap.dtype) != 2:
                    ap = ap.bitcast(mybir.dt.bfloat16)
                sl = [slice(0, 1)] * len(ap.shape)
                for d in range(len(ap.shape) - 1, 0, -1):
                    if ap.shape[d] >= 2:
                        sl[d] = slice(0, 2)
                        break
                nc.tensor.ldweights(ap[tuple(sl)])

            for t in (wkq8, wvp8, ones5):
                pe_touch(t)
            for t in (gsel[0], gsel[1], bsel, ones_row, ones_col, bp_row):
                pe_touch(t)

            # Let the DVE observe the small-constant DMA queues early.
            for t in (gnw[0], gnw[1], gnb[0], gnb[1]):
                dvt = sb.tile([128, 1], FP32, tag="dvt", bufs=1, name="dvt")
                nc.vector.tensor_copy(out=dvt, in_=t)

            # ---------------- group-norm statistics ------------------------
            stat = [sb.tile([128, 2], FP32, tag=f"stat{cb}", name=f"stat{cb}") for cb in range(CB)]
            SQCH = 1024
            sums = [sb.tile([128, 4], FP32, tag=f"sums{cb}", bufs=1, name="sums") for cb in range(CB)]
            sqas = [sb.tile([128, N // SQCH], FP32, tag=f"sqa{cb}", bufs=1, name="sqa") for cb in range(CB)]
            for h in range(4):
                for cb in range(CB):
                    nc.vector.reduce_sum(
                        sums[cb][:, h : h + 1],
                        xs[cb][:, h * XH : (h + 1) * XH],
                        axis=mybir.AxisListType.X,
                    )
                    scr = sb.tile([128, SQCH], FP32, tag="sq_scratch", bufs=2, name="scr")
                    nc.scalar.activation(
                        out=scr, in_=xs[cb][:, h * SQCH : (h + 1) * SQCH],
                        func=Act.Square, accum_out=sqas[cb][:, h : h + 1],
                    )
            for cb in range(CB):
                nc.vector.reduce_sum(stat[cb][:, 0:1], sums[cb], axis=mybir.AxisListType.X)
                nc.vector.reduce_sum(stat[cb][:, 1:2], sqas[cb], axis=mybir.AxisListType.X)

            gstats_ps = ps.tile([G, 2], FP32, tag="pp", bufs=3, name="gstats_ps")
            for cb in range(CB):
                nc.tensor.matmul(
                    gstats_ps, lhsT=gsel[cb], rhs=stat[cb],
                    start=(cb == 0), stop=(cb == CB - 1),
                )
            m2 = sb.tile([G, 2], FP32, tag="m2")
            nc.vector.tensor_scalar_mul(out=m2, in0=gstats_ps, scalar1=INV_CNT)
            meansq = sb.tile([G, 1], FP32, tag="meansq")
            nc.vector.tensor_mul(out=meansq, in0=m2[:, 0:1], in1=m2[:, 0:1])
            gm = sb.tile([G, 2], FP32, tag="gm")
            nc.vector.tensor_sub(out=gm[:, 1:2], in0=m2[:, 1:2], in1=meansq)
            eps_t = sb.tile([G, 1], FP32, tag="eps_t")
            nc.vector.memset(eps_t, EPS)
            nc.scalar.activation(out=gm[:, 1:2], in_=gm[:, 1:2], func=Act.Sqrt, bias=eps_t)
            nc.vector.reciprocal(out=gm[:, 1:2], in_=gm[:, 1:2])
            nc.vector.tensor_copy(out=gm[:, 0:1], in_=m2[:, 0:1])
            pe_touch(gm)

            scale_v = []
            bias_v = []
            for cb in range(CB):
                bvals_ps = ps.tile([128, 2], FP32, tag="pp", bufs=3, name="bvals_ps")
                nc.tensor.matmul(
                    bvals_ps, lhsT=bsel[:, cb * 128 : (cb + 1) * 128], rhs=gm,
                    start=True, stop=True,
                )
                sc = sb.tile([128, 1], FP32, tag=f"scale{cb}", name=f"scale{cb}")
                bi = sb.tile([128, 1], FP32, tag=f"bias{cb}", name=f"bias{cb}")
                tmp = sb.tile([128, 1], FP32, tag=f"tmpb{cb}", name=f"tmpb{cb}")
                # sc = rstd * gn_w/4 ; bi = gn_b/4 - mean * sc
                nc.vector.tensor_mul(out=sc, in0=bvals_ps[:, 1:2], in1=gnw[cb])
                nc.vector.tensor_mul(out=tmp, in0=bvals_ps[:, 0:1], in1=sc)
                nc.vector.tensor_sub(out=bi, in0=gnb[cb], in1=tmp)
                scale_v.append(sc)
                bias_v.append(bi)

            # ---------------- xn8 / g8 / vp (phase B) ----------------------
            xn8 = sb.tile([128, 2, N], E4, tag="xn8")
            g8 = sb.tile([128, 2, N], E4, tag="g8")
            vpp = [
                sb.tile([128, 2, C], E4, tag="vpp", bufs=NP, name=f"vpp{m}")
                for m in range(NP)
            ]

            BC = 1024  # big-chunk width for phase B
            for bc in range(N // BC):
                nsl = slice(bc * BC, (bc + 1) * BC)
                # xn8 = x*sc + bi: cb0 on ACT (Identity), cb1 on DVE
                # (tensor_scalar) so the halves run in parallel; Pool can't
                # help -- TensorScalarPtr is not a valid Pool opcode.
                nc.scalar.activation(
                    out=xn8[:, 0, nsl], in_=xs[0][:, nsl], func=Act.Identity,
                    bias=bias_v[0], scale=scale_v[0],
                )
                nc.vector.tensor_scalar(
                    out=xn8[:, 1, nsl], in0=xs[1][:, nsl],
                    scalar1=scale_v[1], scalar2=bias_v[1],
                    op0=Alu.mult, op1=Alu.add,
                )
                # g = M xn + bg  (one DR matmul + conv per 512-half, out of
                # the 1-bank "pp" ring; convs alternate ACT/DVE)
                for ob in range(CB):
                    osl = slice(ob * 128, (ob + 1) * 128)
                    for hh in range(2):
                        hsl = slice(bc * BC + hh * IC, bc * BC + (hh + 1) * IC)
                        gp = ps.tile([128, IC], FP32, tag="pp", bufs=3, name="gp")
                        nc.tensor.matmul(
                            gp, lhsT=wkq8[:, :, osl], rhs=xn8[:, :, hsl],
                            start=True, stop=True, perf_mode=DR,
                        )
                        if (ob + hh) % 2 == 0:
                            nc.scalar.activation(
                                out=g8[:, ob, hsl], in_=gp,
                                func=Act.Identity, bias=bgt[ob],
                            )
                        else:
                            nc.vector.tensor_scalar_add(
                                out=g8[:, ob, hsl], in0=gp, scalar1=bgt[ob],
                            )
                # vp = Wvp4 xn8 per 128-j block; pairs packed for DR AV
                for mm_i in range(4):
                    m = bc * 4 + mm_i
                    for i2 in range(2):
                        jb = 2 * m + i2
                        jsl = slice(jb * 128, (jb + 1) * 128)
                        vpm = ps.tile([128, C], FP32, tag="pp", bufs=3, name="vpm")
                        nc.tensor.matmul(
                            vpm, lhsT=xn8[:, :, jsl], rhs=wvp8,
                            start=True, stop=True, perf_mode=DR,
                        )
                        if (mm_i + i2) % 2 == 0:
                            nc.vector.tensor_copy(out=vpp[m][:, i2, :], in_=vpm)
                        else:
                            nc.scalar.copy(out=vpp[m][:, i2, :], in_=vpm)

            # ---------------- attention (phase C) --------------------------
            LAG = 2
            pending = []
            for ich in range(NI):
                isl = slice(ich * IC, (ich + 1) * IC)

                pp_ps = [
                    ps.tile([128, IC], FP32, tag="pp", bufs=3, name=f"pp{cb}_{ich}")
                    for cb in range(CB)
                ]
                z_ps = ps.tile([2, IC], FP32, tag="pp", bufs=3, name=f"z{ich}")

                ets = [None] * NP

                def issue_st(m):
                    # single-bank score tiles in a 5-deep ring: the PE can
                    # run 2.5 pairs ahead of the exp engines instead of 2
                    et = sb.tile([128, 2, IC], E5, tag="et", bufs=6, name=f"et{m}")
                    et_u8 = et.bitcast(U8)
                    for i2 in range(2):
                        jb = 2 * m + i2
                        jsl = slice(jb * 128, (jb + 1) * 128)
                        stp = ps.tile([128, IC], FP32, tag="mm", bufs=5, name="stp")
                        nc.tensor.matmul(
                            stp, lhsT=xn8[:, :, jsl], rhs=g8[:, :, isl],
                            start=True, stop=True, perf_mode=DR,
                        )
                        # alternate which engine takes which half per pair;
                        # pair 5 goes fully to DVE (ACT 15 / DVE 17 balance:
                        # ACT also carries the zs/ppc/zbs tail copies)
                        if (m + i2) % 2 == 0 and m != 5:
                            nc.scalar.activation(
                                out=et[:, i2, :], in_=stp, func=Act.Exp,
                            )
                        else:
                            nc.vector.tensor_scalar(
                                out=et_u8[:, i2, :], in0=stp,
                                scalar1=SCH_A, scalar2=SCH_B,
                                op0=Alu.mult, op1=Alu.add,
                            )
                    ets[m] = et

                def issue_av(m):
                    et = ets[m]
                    for cb in range(CB):
                        # with bias, the pp group is closed by the bp*Z matmul
                        nc.tensor.matmul(
                            pp_ps[cb], lhsT=vpp[m][:, :, cb * 128 : (cb + 1) * 128],
                            rhs=et, start=(m == 0),
                            stop=(not has_bp and m == NP - 1),
                            perf_mode=DR,
                        )
                    nc.tensor.matmul(
                        z_ps, lhsT=ones5[:, :, 0:2], rhs=et,
                        start=(m == 0), stop=(m == NP - 1), perf_mode=DR,
                    )

                for m in range(NP + LAG):
                    for fm, fn in pending:
                        if fm == m:
                            fn()
                    if m < NP:
                        issue_st(m)
                    if m >= LAG:
                        issue_av(m - LAG)
                pending = []

                last = ich == NI - 1

                def make_tails(ich=ich, isl=isl, pp_ps=pp_ps,
                               z_ps=z_ps, last=last):
                    state = {}

                    def tail_early():
                        # Z copy out of PSUM; reciprocal runs on a DMA-
                        # reshaped [128, 4] view so the 6-pass DVE reciprocal
                        # costs ~0.2us instead of 3us on [1, 512].  The last
                        # chunk takes the direct lower-latency reciprocal.
                        zs = sb.tile([1, IC], FP32R, tag="zs", bufs=2, name="zs")
                        nc.scalar.copy(out=zs, in_=z_ps[0:1, :])
                        state["zs"] = zs
                        if not last:
                            zt = sb.tile([128, 4], FP32R, tag="zt", bufs=2, name="zt")
                            nc.sync.dma_start(out=zt, in_=zs)
                            state["zt"] = zt
                        # bias (when nonzero) enters pre-normalization:
                        # pp += bp * Z, so pp/Z carries +bp.  These rank-1
                        # fp32r matmuls also close the pp accumulation groups.
                        if has_bp:
                            for ob in range(CB):
                                nc.tensor.matmul(
                                    pp_ps[ob],
                                    lhsT=bp_row[:, ob * 128 : (ob + 1) * 128],
                                    rhs=state["zs"], start=False, stop=True,
                                )
                        # both ppc copies run early: with z sharing the pp
                        # ring there is no spare slot, so the next chunk's
                        # first AV needs both accumulators drained by m==2
                        ppcs = []
                        for ob in range(CB):
                            ppc = sb.tile([128, IC], FP32, tag="ppc", bufs=3, name="ppc")
                            nc.scalar.copy(out=ppc, in_=pp_ps[ob])
                            ppcs.append(ppc)
                        state["ppcs"] = ppcs

                    def tail_recip():
                        # placed a few pairs into the next chunk so the
                        # zs->zt DMA has landed and DVE doesn't stall
                        zrr = sb.tile([1, IC], FP32R, tag="zrr", bufs=2, name="zrr")
                        if last:
                            nc.vector.reciprocal(out=zrr, in_=z_ps[0:1, :])
                        else:
                            ztr = sb.tile([128, 4], FP32R, tag="ztr", bufs=2, name="ztr")
                            nc.vector.reciprocal(out=ztr, in_=state["zt"])
                            nc.sync.dma_start(out=zrr, in_=ztr)
                        state["zrr"] = zrr

                    def tail_late():
                        zb_ps = ps.tile([128, IC], FP32, tag="mm", bufs=5, name="zb")
                        nc.tensor.matmul(
                            zb_ps, lhsT=ones_row, rhs=state["zrr"],
                            start=True, stop=True,
                        )
                        zbs = sb.tile([128, IC], FP32, tag="zbs", bufs=2, name="zbs")
                        nc.scalar.copy(out=zbs, in_=zb_ps)
                        for ob in range(CB):
                            osl = slice(ob * 128, (ob + 1) * 128)
                            t = sb.tile([128, IC], FP32, tag="tn", bufs=2, name="tn")
                            nc.gpsimd.tensor_mul(out=t, in0=state["ppcs"][ob], in1=zbs)
                            fin = sb.tile([128, IC], FP32, tag="fin", bufs=3, name="fin")
                            nc.gpsimd.tensor_add(out=fin, in0=t, in1=xs[ob][:, isl])
                            nc.sync.dma_start(out=y_d[osl, isl], in_=fin)

                    return [(0, tail_early), (3, tail_recip), (6, tail_late)]

                pending = make_tails()
            for _, fn in pending:
                fn()

    return nc


def _prep_inputs(x_full, gn_w, gn_b, wq, bq, wk, bk, wv, bv, wp, bp):
    f = np.float32
    f64 = np.float64
    M = (np.asarray(wk, f64).T @ np.asarray(wq, f64)).astype(f)
    Wvp4 = (4.0 * (np.asarray(wp, f64) @ np.asarray(wv, f64))).astype(f)
    bg4 = ((np.asarray(wk, f64).T @ np.asarray(bq, f64)) / 4.0).astype(f).reshape(C, 1)
    bp2 = (np.asarray(bp, f64) + np.asarray(wp, f64) @ np.asarray(bv, f64)
           ).astype(f).reshape(C, 1)

    def dr_pack(mat):
        # [C, C] weight (contraction dim first) -> [128, 2, C] DoubleRow tile
        return np.ascontiguousarray(
            mat.reshape(2, 128, C).transpose(1, 0, 2)
        ).astype(ml_dtypes.float8_e4m3)

    # g[o, n] = sum_c M[o, c] xn[c, n]  ->  lhsT[p, blk, o] = M.T[blk*128+p, o]
    wkq8 = dr_pack(np.ascontiguousarray(M.T))
    wvp8 = dr_pack(np.ascontiguousarray(Wvp4.T))

    gnw4 = (np.asarray(gn_w, f) / 4.0).reshape(C, 1)
    gnb4 = (np.asarray(gn_b, f) / 4.0).reshape(C, 1)
    gsel = np.zeros((C, G), f)
    for c in range(C):
        gsel[c, c // GS] = 1.0
    bsel = np.ascontiguousarray(gsel.T)

    shared = dict(
        wkq8=wkq8, wvp8=wvp8, bg4=bg4, bp2=bp2,
        gnw4=gnw4, gnb4=gnb4, gsel=gsel, bsel=bsel,
        ones5=np.ones((128, 2, 16), ml_dtypes.float8_e5m2),
        ones_row=np.ones((1, 128), f),
        ones_col=np.ones((128, 2), f),
        bp_row=np.ascontiguousarray(bp2.reshape(1, C)),
    )
    in_maps = []
    for b in range(B):
        m = dict(shared)
        m["xbf"] = np.ascontiguousarray(
            x_full[b].reshape(C, N).astype(ml_dtypes.bfloat16)
        )
        in_maps.append(m)
    return in_maps


_CACHED_NC = {}


def _get_nc(has_bp: bool = False):
    if has_bp not in _CACHED_NC:
        _CACHED_NC[has_bp] = build_bass(has_bp)
    return _CACHED_NC[has_bp]


def kernel(x, gn_w, gn_b, wq, bq, wk, bk, wv, bv, wp, bp):
    from concourse.bass_utils import run_bass_kernel_spmd

    in_maps = _prep_inputs(
        np.asarray(x), np.asarray(gn_w), np.asarray(gn_b),
        np.asarray(wq), np.asarray(bq), np.asarray(wk), np.asarray(bk),
        np.asarray(wv), np.asarray(bv), np.asarray(wp), np.asarray(bp),
    )
    nc = _get_nc(has_bp=bool(np.any(in_maps[0]["bp_row"])))
    res = run_bass_kernel_spmd(nc, in_maps, list(range(B)))
    out = np.empty((B, C, H, W), np.float32)
    for b in range(B):
        out[b] = res.results[b]["y"].reshape(C, H, W)
    return out
